# revision 3
# baseline (speedup 1.0000x reference)
"""Trainium2 Bass kernel v2 for nn_BrainGeneratorModel.

8 cores = (sample b 0..3) x (H-half 0..1); per core: slab [D=192, HS=120, W=192]
-> img [192, 96, 192] f32 + label remap of [128, 27648] i16.

Image pipeline (bf16 matmuls, banded split => each blur one 128-contraction):
  A) per d-batch (DB=8): bias matmul (K=4) -> exp (ACT, bf16) -> x*exp (DVE)
     -> H-blur (K=120) -> PE transpose w onto partitions (rows 0:128 & 64:192)
     -> W-blur (banded halves) -> y2 DRAM bf16 [w', d, h'].
  B) load y2 rows whole; per hl-batch: PE transpose d onto partitions (strided
     moving) -> D-blur (banded halves) -> img f32.

Labels (bit-plane, chunks of FC): v = 2^l via f32 exponent-encode;
per output bit b: pb = ((v & mask_b) != 0) * 2^b; sequential i16 accumulate.
Masks arrive via a [128, 8] u32 input tile (exact 32-bit values).
"""

import sys

for _p in ("/opt/trn_rl_repo",):
    if _p not in sys.path:
        sys.path.insert(0, _p)

import numpy as np
import ml_dtypes

import concourse.bass as bass
import concourse.mybir as mybir
import concourse.bacc as bacc
import concourse.tile as tile
from concourse.bass_utils import run_bass_kernel_spmd

F32 = mybir.dt.float32
BF16 = mybir.dt.bfloat16
I16 = mybir.dt.int16
I32 = mybir.dt.int32
U32 = mybir.dt.uint32
ALU = mybir.AluOpType

B, C, D, H, W = 4, 1, 192, 192, 192
SMALL = 4
BIAS_STD = 0.7
MAX_SIGMA = 3.0
TRUNCATE = 4.0
K = 2 * int(TRUNCATE * MAX_SIGMA) + 1  # 25
P = K // 2  # 12
N_LABELS = 32
TABLE = 128

HC = 96            # interior H rows per core
HS = 120           # slab rows
DB = 16            # d-batch size (stage A)
NB_A = D // DB     # 12
FA = DB * W        # 3072
HB = 8             # hl-batch size (stage B)
NB_B = HC // HB    # 12
FLAB = D * HC * W // 128  # 27648
NLC = 18           # label chunks
FC = FLAB // NLC   # 1536

_CACHE = {}


def _lin_weights(n_in, n_out):
    pos = np.linspace(0.0, n_in - 1.0, n_out, dtype=np.float64)
    i0 = np.clip(np.floor(pos).astype(np.int64), 0, n_in - 2)
    f = pos - i0
    Wm = np.zeros((n_out, n_in), np.float64)
    r = np.arange(n_out)
    np.add.at(Wm, (r, i0), 1.0 - f)
    np.add.at(Wm, (r, i0 + 1), f)
    return Wm


def _gauss_kernels(sigma3):
    ar = np.arange(K, dtype=np.float64) - K // 2
    out = np.zeros((3, K), np.float64)
    for i, sg in enumerate(sigma3):
        s = max(float(sg), 1e-3)
        g = np.exp(-0.5 * ar * ar / (s * s))
        g = g / g.sum()
        if float(sg) >= 0.01:
            out[i] = g
        else:
            out[i, K // 2] = 1.0
    return out


def _edge_folded_toeplitz(g, n):
    M = np.zeros((n, n), np.float64)
    for j in range(n):
        for t in range(K):
            src = min(max(j + t - P, 0), n - 1)
            M[src, j] += g[t]
    return M


def _slab_toeplitz(g):
    M = np.zeros((HS, HC), np.float64)
    for j in range(HC):
        for t in range(K):
            M[j + t, j] += g[t]
    return M


def _imm_i32(m):
    """Signed-int representation of a u32 mask for exact f64 imm transport."""
    m = int(m) & 0xFFFFFFFF
    return float(m - (1 << 32) if m >= (1 << 31) else m)


def _emit_label_chunk(nc, ltp, labp, labq, lab_h, labo_h, cc, masks, pending,
                      carry):
    """One label chunk [128, FC]: bit-plane remap of host-provided one-hot
    (vu = 1 << label, u32), i16 out.

    Pool sums pb1+pb2 and pb3+pb4; the DVE merge of those partials is
    deferred to the NEXT chunk (so DVE never waits on Pool), as is the
    output DMA (so SP never waits on DVE)."""
    while pending:
        po, pacc = pending.pop(0)
        nc.sync.dma_start(po, pacc)
    sl = slice(cc * FC, (cc + 1) * FC)
    vu = ltp.tile([128, FC], U32, tag="lt")
    nc.sync.dma_start(vu[:], lab_h.ap()[:, sl])

    def mk_pb(b, tag):
        eb = labp.tile([128, FC], U32, tag="eb")
        nc.vector.tensor_scalar(eb[:], vu[:], _imm_i32(masks[b]), None,
                                ALU.bitwise_and)
        pb = labp.tile([128, FC], I16, tag=tag)
        nc.vector.tensor_scalar(pb[:], eb[:], 0.0, float(2 ** b),
                                ALU.not_equal, ALU.mult)
        return pb

    acc = labq.tile([128, FC], I16, tag="acc")
    pb0 = mk_pb(0, "pbA")
    pb1 = mk_pb(1, "pbB")
    nc.vector.tensor_tensor(acc[:], pb0[:], pb1[:], ALU.add)
    for b in range(2, 7):
        pbx = mk_pb(b, "pbA" if b % 2 == 0 else "pbB")
        nc.vector.tensor_tensor(acc[:], acc[:], pbx[:], ALU.add)
    pending.append((labo_h.ap()[:, slice(cc * FC, (cc + 1) * FC)], acc[:]))


def _flush_label_carry(nc, labo_h, pending, carry):
    return


def _build_program(masks):
    nc = bacc.Bacc("TRN2", target_bir_lowering=False, debug=False)

    # ---- external inputs (per core) ----
    xs_h = nc.dram_tensor("xs", [D, HS, W], BF16, kind="ExternalInput")
    c_h = nc.dram_tensor("cydw", [4, D * W], BF16, kind="ExternalInput")
    wht_h = nc.dram_tensor("wht", [4, HS], BF16, kind="ExternalInput")
    gh_h = nc.dram_tensor("gh", [HS, HC], BF16, kind="ExternalInput")
    gwa_h = nc.dram_tensor("gwa", [128, 96], BF16, kind="ExternalInput")
    gwc_h = nc.dram_tensor("gwc", [128, 96], BF16, kind="ExternalInput")
    gda_h = nc.dram_tensor("gda", [128, 96], BF16, kind="ExternalInput")
    gdc_h = nc.dram_tensor("gdc", [128, 96], BF16, kind="ExternalInput")
    id_h = nc.dram_tensor("idm", [128, 128], BF16, kind="ExternalInput")
    lab_h = nc.dram_tensor("lab", [128, FLAB], U32, kind="ExternalInput")

    # ---- external outputs ----
    img_h = nc.dram_tensor("img", [D, HC, W], F32, kind="ExternalOutput")
    labo_h = nc.dram_tensor("labo", [128, FLAB], I16, kind="ExternalOutput")

    with tile.TileContext(nc) as tc:
        from contextlib import ExitStack
        with ExitStack() as _stk:
            cst = _stk.enter_context(tc.tile_pool(name="consts", bufs=1))
            sxp = _stk.enter_context(tc.tile_pool(name="sxp", bufs=3))
            cbp = _stk.enter_context(tc.tile_pool(name="cbp", bufs=2))
            ebp = _stk.enter_context(tc.tile_pool(name="ebp", bufs=8))
            xhp = _stk.enter_context(tc.tile_pool(name="xhp", bufs=2))
            zwp = _stk.enter_context(tc.tile_pool(name="zwp", bufs=2))
            zvp = _stk.enter_context(tc.tile_pool(name="zvp", bufs=2))
            ybp = _stk.enter_context(tc.tile_pool(name="ybp", bufs=1))
            zdp = _stk.enter_context(tc.tile_pool(name="zdp", bufs=2))
            zip_ = _stk.enter_context(tc.tile_pool(name="zip", bufs=2))
            ltp = _stk.enter_context(tc.tile_pool(name="ltp", bufs=2))
            labp = _stk.enter_context(tc.tile_pool(name="labp", bufs=1))
            labq = _stk.enter_context(tc.tile_pool(name="labq", bufs=2))
            psbp = _stk.enter_context(tc.tile_pool(name="psb", bufs=2, space="PSUM"))
            pshp = psbp
            pstp = _stk.enter_context(tc.tile_pool(name="pst", bufs=2, space="PSUM"))
            pswp = _stk.enter_context(tc.tile_pool(name="psw", bufs=2, space="PSUM"))
            drp = _stk.enter_context(tc.tile_pool(name="dram", bufs=1, space="DRAM"))
            # ---- constants ----
            ght = cst.tile([HS, HC], BF16)
            nc.sync.dma_start(ght[:], gh_h.ap())
            gwa = cst.tile([128, 96], BF16)
            nc.sync.dma_start(gwa[:], gwa_h.ap())
            gwc = cst.tile([128, 96], BF16)
            nc.sync.dma_start(gwc[:], gwc_h.ap())
            gda = cst.tile([128, 96], BF16)
            nc.sync.dma_start(gda[:], gda_h.ap())
            gdc = cst.tile([128, 96], BF16)
            nc.sync.dma_start(gdc[:], gdc_h.ap())
            whtt = cst.tile([4, HS], BF16)
            nc.sync.dma_start(whtt[:], wht_h.ap())
            idt = cst.tile([128, 128], BF16)
            nc.sync.dma_start(idt[:], id_h.ap())

            # y2 SBUF-resident: [w'-half, (d, h')] bf16, written by stage A
            ybA = ybp.tile([96, D * HC], BF16)
            ybC = ybp.tile([96, D * HC], BF16)

            lab_done = 0
            pending = []
            pending_img = []
            carry = []
            # prefetch two label chunks before stage A (DVE idle at startup)
            for _ in range(2):
                _emit_label_chunk(nc, ltp, labp, labq, lab_h, labo_h,
                                  lab_done, masks, pending, carry)
                lab_done += 1

            # ================= stage A =================
            for ib in range(NB_A):
                d0 = ib * DB
                sx = sxp.tile([HS, FA], BF16, tag="sx")
                nc.sync.dma_start(
                    sx[:],
                    bass.AP(xs_h, d0 * HS * W, [[W, HS], [HS * W, DB], [1, W]]),
                )
                cb = cbp.tile([4, FA], BF16, tag="cb")
                nc.sync.dma_start(cb[:], c_h.ap()[:, d0 * W:(d0 + DB) * W])

                xh = xhp.tile([HC, FA], BF16, tag="xh")
                NQ = FA // 512
                ebs_ = []
                for q in range(NQ):
                    sl = slice(q * 512, (q + 1) * 512)
                    psb = psbp.tile([HS, 512], F32, tag="psbh")
                    nc.tensor.matmul(psb[:], whtt[:], cb[:, sl], start=True, stop=True)
                    eb = ebp.tile([HS, 512], BF16, tag="eb")
                    nc.scalar.activation(eb[:], psb[:], mybir.ActivationFunctionType.Exp)
                    ebs_.append(eb)
                for q in range(NQ):
                    sl = slice(q * 512, (q + 1) * 512)
                    nc.gpsimd.tensor_tensor(sx[:, sl], sx[:, sl], ebs_[q][:], ALU.mult)
                pshs = []
                for q in range(NQ):
                    sl = slice(q * 512, (q + 1) * 512)
                    psh = pshp.tile([HC, 512], F32, tag="psbh")
                    nc.tensor.matmul(psh[:], ght[:], sx[:, sl], start=True, stop=True)
                    pshs.append(psh)
                for q in range(NQ):
                    sl = slice(q * 512, (q + 1) * 512)
                    nc.scalar.copy(xh[:, sl], pshs[q][:])

                # T1: w onto partitions; zwA rows 0..127, zwC rows 64..191
                # layout [128, (dl, h')]
                zwA = zwp.tile([128, DB * HC], BF16, tag="zwA")
                zwC = zwp.tile([128, DB * HC], BF16, tag="zwC")
                for g in range(DB // 8):
                    ptA = pstp.tile([128, 8 * HC], BF16, tag="pt")
                    ptC = pstp.tile([128, 8 * HC], BF16, tag="pt")
                    for t in range(8):
                        dl = g * 8 + t
                        nc.tensor.transpose(
                            ptA[:, t * HC:(t + 1) * HC],
                            xh[:, dl * W: dl * W + 128], idt[0:HC, 0:HC])
                        nc.tensor.transpose(
                            ptC[:, t * HC:(t + 1) * HC],
                            xh[:, dl * W + 64: dl * W + 192], idt[0:HC, 0:HC])
                    nc.scalar.copy(zwA[:, g * 8 * HC:(g + 1) * 8 * HC], ptA[:])
                    nc.scalar.copy(zwC[:, g * 8 * HC:(g + 1) * 8 * HC], ptC[:])

                # W-blur: m=0 from zwA (w rows 0..127), m=1 from zwC (64..191)
                for m, (gw_, zw_, yb_) in enumerate(((gwa, zwA, ybA), (gwc, zwC, ybC))):
                    for q in range(DB * HC // 512):
                        sl = slice(d0 * HC + q * 512, d0 * HC + (q + 1) * 512)
                        psw = pswp.tile([96, 512], F32, tag="psw")
                        nc.tensor.matmul(psw[:], gw_[:], zw_[:, q * 512:(q + 1) * 512],
                                         start=True, stop=True)
                        nc.scalar.copy(yb_[:, sl], psw[:])

                if lab_done < 2 + NB_A:
                    _emit_label_chunk(nc, ltp, labp, labq, lab_h, labo_h,
                                      lab_done, masks, pending, carry)
                    lab_done += 1

            # ================= stage B ================= (yb tiles already filled)

            for jb in range(NB_B):
                h0 = jb * HB
                # T2: d onto partitions; strided moving reads from yb tiles
                # zdA rows d 0..127, zdC rows d 64..191; layout [*, (hl, w)]
                zdA = zdp.tile([128, HB * W], BF16, tag="zdA")
                zdC = zdp.tile([128, HB * W], BF16, tag="zdC")
                for g in range(HB // 4):
                    ptA = pstp.tile([128, 768], BF16, tag="pt")
                    ptC = pstp.tile([128, 768], BF16, tag="pt")
                    for t in range(4):
                        hl = g * 4 + t
                        colA = bass.AP(ybA.tensor, ybA[:].offset + (h0 + hl),
                                       [ybA[:].ap[0], [HC, 128]])
                        colB = bass.AP(ybC.tensor, ybC[:].offset + (h0 + hl),
                                       [ybC[:].ap[0], [HC, 128]])
                        nc.tensor.transpose(ptA[:, t * 192 + 0:t * 192 + 96],
                                            colA, idt[0:96, 0:96])
                        nc.tensor.transpose(ptA[:, t * 192 + 96:t * 192 + 192],
                                            colB, idt[0:96, 0:96])
                        colA2 = bass.AP(ybA.tensor, ybA[:].offset + (h0 + hl) + 64 * HC,
                                        [ybA[:].ap[0], [HC, 128]])
                        colB2 = bass.AP(ybC.tensor, ybC[:].offset + (h0 + hl) + 64 * HC,
                                        [ybC[:].ap[0], [HC, 128]])
                        nc.tensor.transpose(ptC[:, t * 192 + 0:t * 192 + 96],
                                            colA2, idt[0:96, 0:96])
                        nc.tensor.transpose(ptC[:, t * 192 + 96:t * 192 + 192],
                                            colB2, idt[0:96, 0:96])
                    nc.scalar.copy(zdA[:, g * 768:(g + 1) * 768], ptA[:])
                    nc.scalar.copy(zdC[:, g * 768:(g + 1) * 768], ptC[:])

                # D-blur: n=0 -> img rows 0..95 from zdA; n=1 -> 96..191 from zdC
                for n, (gd_, zd_) in enumerate(((gda, zdA), (gdc, zdC))):
                    zi = zip_.tile([96, HB * W], F32, tag="zi")
                    for q in range(HB * W // 512):
                        sl = slice(q * 512, (q + 1) * 512)
                        psd = pswp.tile([96, 512], F32, tag="psw")
                        nc.tensor.matmul(psd[:], gd_[:], zd_[:, sl], start=True, stop=True)
                        nc.scalar.copy(zi[:, sl], psd[:])
                    pending_img.append((
                        bass.AP(img_h, n * 96 * HC * W + h0 * W,
                                [[HC * W, 96], [1, HB * W]]),
                        zi[:],
                    ))
                while len(pending_img) > 2:
                    po, pz = pending_img.pop(0)
                    nc.sync.dma_start(po, pz)

                if jb % 3 == 0 and lab_done < NLC:
                    _emit_label_chunk(nc, ltp, labp, labq, lab_h, labo_h,
                                      lab_done, masks, pending, carry)
                    lab_done += 1

            while lab_done < NLC:
                _emit_label_chunk(nc, ltp, labp, labq, lab_h, labo_h,
                                  lab_done, masks, pending, carry)
                lab_done += 1
            _flush_label_carry(nc, labo_h, pending, carry)
            while pending:
                po, pacc = pending.pop(0)
                nc.sync.dma_start(po, pacc)
            while pending_img:
                po, pz = pending_img.pop(0)
                nc.sync.dma_start(po, pz)

    nc.compile()
    return nc


def _host_prep(x, small_bias, sigma01, labels, source_values, dest_values):
    Wd = _lin_weights(SMALL, D)
    Whm = _lin_weights(SMALL, H)
    Wwm = _lin_weights(SMALL, W)
    eye_bf = np.eye(128, dtype=ml_dtypes.bfloat16)

    mapping = np.zeros(TABLE, np.int64)
    mapping[np.asarray(source_values, np.int64)] = np.asarray(dest_values, np.int64)

    x_np = np.asarray(x, np.float32)
    lab_np = np.asarray(labels)

    in_maps = []
    for c in range(8):
        b, half = c // 2, c % 2
        h0 = half * HC
        hidx = np.clip(np.arange(h0 - P, h0 + HC + P), 0, H - 1)

        xs = np.ascontiguousarray(x_np[b, 0][:, hidx, :]).astype(ml_dtypes.bfloat16)

        sm = np.asarray(small_bias[b, 0], np.float64) * BIAS_STD
        Cydw = np.einsum("xyz,dx,wz->ydw", sm, Wd, Wwm).reshape(4, D * W)
        WhT = np.ascontiguousarray(Whm[hidx, :].T)

        g3 = _gauss_kernels(np.asarray(sigma01[b], np.float64) * MAX_SIGMA)
        Gh = _slab_toeplitz(g3[1])
        Gw = _edge_folded_toeplitz(g3[2], W)
        Gd = _edge_folded_toeplitz(g3[0], D)

        lab = (np.uint32(1) << lab_np[b, 0][:, h0:h0 + HC, :].astype(np.uint32)
               ).reshape(128, FLAB)

        in_maps.append({
            "xs": xs,
            "cydw": Cydw.astype(ml_dtypes.bfloat16),
            "wht": WhT.astype(ml_dtypes.bfloat16),
            "gh": Gh.astype(ml_dtypes.bfloat16),
            "gwa": np.ascontiguousarray(Gw[0:128, 0:96]).astype(ml_dtypes.bfloat16),
            "gwc": np.ascontiguousarray(Gw[64:192, 96:192]).astype(ml_dtypes.bfloat16),
            "gda": np.ascontiguousarray(Gd[0:128, 0:96]).astype(ml_dtypes.bfloat16),
            "gdc": np.ascontiguousarray(Gd[64:192, 96:192]).astype(ml_dtypes.bfloat16),
            "idm": eye_bf,
            "lab": np.ascontiguousarray(lab),
        })
    return in_maps


def _label_masks(source_values, dest_values):
    mapping = np.zeros(TABLE, np.int64)
    mapping[np.asarray(source_values, np.int64)] = np.asarray(dest_values, np.int64)
    masks = []
    for b in range(7):
        m = 0
        for l in range(N_LABELS):
            if (int(mapping[l]) >> b) & 1:
                m |= 1 << l
        masks.append(m)
    return tuple(masks)


def kernel(x, small_bias, sigma01, labels, source_values, dest_values):
    masks = _label_masks(source_values, dest_values)
    if _CACHE.get("masks") != masks:
        _CACHE["nc"] = _build_program(masks)
        _CACHE["masks"] = masks
    nc = _CACHE["nc"]

    in_maps = _host_prep(x, small_bias, sigma01, labels, source_values, dest_values)
    res = run_bass_kernel_spmd(nc, in_maps, core_ids=list(range(8)))

    img = np.empty((B, C, D, H, W), np.float32)
    labels_out = np.empty((B, C, D, H, W), np.int32)
    for c in range(8):
        b, half = c // 2, c % 2
        h0 = half * HC
        r = res.results[c]
        img[b, 0, :, h0:h0 + HC, :] = r["img"].reshape(D, HC, W)
        labels_out[b, 0, :, h0:h0 + HC, :] = (
            r["labo"].reshape(D, HC, W).astype(np.int32))
    return img, labels_out


# revision 5
# speedup vs baseline: 1.0142x; 1.0142x over previous
"""Trainium2 Bass kernel v2 for nn_BrainGeneratorModel.

8 cores = (sample b 0..3) x (H-half 0..1); per core: slab [D=192, HS=120, W=192]
-> img [192, 96, 192] f32 + label remap of [128, 27648] i16.

Image pipeline (bf16 matmuls, banded split => each blur one 128-contraction):
  A) per d-batch (DB=16): bias matmul (K=4) -> exp (ACT, bf16) -> x*exp
     (Pool, in place) -> H-blur (K=120) -> PE transpose w onto partitions
     (overlapping rows 0:128 & 64:192) -> W-blur (banded halves) -> SBUF-
     resident y2 [w'-half, (d, h')] bf16 (no DRAM round trip).
  B) per hl-batch: PE transpose d onto partitions (strided moving reads of
     y2) -> D-blur (banded halves) -> img f32 (delayed-issue DMA).

Labels (bit-plane, 18 chunks, pure DVE): host ships vu = 1 << label (u32);
per output bit b: pb = ((vu & mask_b) != 0) * 2^b via one bitwise AND (mask
as exact integer immediate) + one fused not_equal/mult; six i16 adds fold
the planes. Output DMAs are issued one chunk late so SP never head-of-line
blocks; chunks are interleaved through both stages with two prefetched.
"""

import sys

for _p in ("/opt/trn_rl_repo",):
    if _p not in sys.path:
        sys.path.insert(0, _p)

import numpy as np
import ml_dtypes

import concourse.bass as bass
import concourse.mybir as mybir
import concourse.bacc as bacc
import concourse.tile as tile
from concourse.bass_utils import run_bass_kernel_spmd

F32 = mybir.dt.float32
BF16 = mybir.dt.bfloat16
I16 = mybir.dt.int16
I32 = mybir.dt.int32
U32 = mybir.dt.uint32
ALU = mybir.AluOpType

B, C, D, H, W = 4, 1, 192, 192, 192
SMALL = 4
BIAS_STD = 0.7
MAX_SIGMA = 3.0
TRUNCATE = 4.0
K = 2 * int(TRUNCATE * MAX_SIGMA) + 1  # 25
P = K // 2  # 12
N_LABELS = 32
TABLE = 128

HC = 96            # interior H rows per core
HS = 120           # slab rows
DB = 16            # d-batch size (stage A)
NB_A = D // DB     # 12
FA = DB * W        # 3072
HB = 8             # hl-batch size (stage B)
NB_B = HC // HB    # 12
FLAB = D * HC * W // 128  # 27648
NLC = 18           # label chunks
FC = FLAB // NLC   # 1536

_CACHE = {}


def _lin_weights(n_in, n_out):
    pos = np.linspace(0.0, n_in - 1.0, n_out, dtype=np.float64)
    i0 = np.clip(np.floor(pos).astype(np.int64), 0, n_in - 2)
    f = pos - i0
    Wm = np.zeros((n_out, n_in), np.float64)
    r = np.arange(n_out)
    np.add.at(Wm, (r, i0), 1.0 - f)
    np.add.at(Wm, (r, i0 + 1), f)
    return Wm


def _gauss_kernels(sigma3):
    ar = np.arange(K, dtype=np.float64) - K // 2
    out = np.zeros((3, K), np.float64)
    for i, sg in enumerate(sigma3):
        s = max(float(sg), 1e-3)
        g = np.exp(-0.5 * ar * ar / (s * s))
        g = g / g.sum()
        if float(sg) >= 0.01:
            out[i] = g
        else:
            out[i, K // 2] = 1.0
    return out


def _edge_folded_toeplitz(g, n):
    M = np.zeros((n, n), np.float64)
    for j in range(n):
        for t in range(K):
            src = min(max(j + t - P, 0), n - 1)
            M[src, j] += g[t]
    return M


def _slab_toeplitz(g):
    M = np.zeros((HS, HC), np.float64)
    for j in range(HC):
        for t in range(K):
            M[j + t, j] += g[t]
    return M


def _imm_i32(m):
    """Signed-int representation of a u32 mask for exact f64 imm transport."""
    m = int(m) & 0xFFFFFFFF
    return float(m - (1 << 32) if m >= (1 << 31) else m)


def _emit_label_chunk(nc, ltp, labp, labq, lab_h, labo_h, cc, masks, pending,
                      carry):
    """One label chunk [128, FC]: bit-plane remap of host-provided one-hot
    (vu = 1 << label, u32), i16 out.

    The output DMA of the PREVIOUS chunk is issued here (its data is long
    ready), so SP never head-of-line blocks on a DVE wait."""
    while pending:
        po, pacc = pending.pop(0)
        nc.sync.dma_start(po, pacc)
    sl = slice(cc * FC, (cc + 1) * FC)
    vu = ltp.tile([128, FC], U32, tag="lt")
    nc.sync.dma_start(vu[:], lab_h.ap()[:, sl])

    def mk_pb(b, tag):
        eb = labp.tile([128, FC], U32, tag="eb")
        nc.vector.tensor_scalar(eb[:], vu[:], _imm_i32(masks[b]), None,
                                ALU.bitwise_and)
        pb = labp.tile([128, FC], I16, tag=tag)
        nc.vector.tensor_scalar(pb[:], eb[:], 0.0, float(2 ** b),
                                ALU.not_equal, ALU.mult)
        return pb

    acc = labq.tile([128, FC], I16, tag="acc")
    pb0 = mk_pb(0, "pbA")
    pb1 = mk_pb(1, "pbB")
    nc.vector.tensor_tensor(acc[:], pb0[:], pb1[:], ALU.add)
    for b in range(2, 7):
        pbx = mk_pb(b, "pbA" if b % 2 == 0 else "pbB")
        nc.vector.tensor_tensor(acc[:], acc[:], pbx[:], ALU.add)
    pending.append((labo_h.ap()[:, slice(cc * FC, (cc + 1) * FC)], acc[:]))


def _flush_label_carry(nc, labo_h, pending, carry):
    return


def _build_program(masks):
    nc = bacc.Bacc("TRN2", target_bir_lowering=False, debug=False)

    # ---- external inputs (per core) ----
    xs_h = nc.dram_tensor("xs", [D, HS, W], BF16, kind="ExternalInput")
    c_h = nc.dram_tensor("cydw", [4, D * W], BF16, kind="ExternalInput")
    wht_h = nc.dram_tensor("wht", [4, HS], BF16, kind="ExternalInput")
    gh_h = nc.dram_tensor("gh", [HS, HC], BF16, kind="ExternalInput")
    gwa_h = nc.dram_tensor("gwa", [128, 96], BF16, kind="ExternalInput")
    gwc_h = nc.dram_tensor("gwc", [128, 96], BF16, kind="ExternalInput")
    gda_h = nc.dram_tensor("gda", [128, 96], BF16, kind="ExternalInput")
    gdc_h = nc.dram_tensor("gdc", [128, 96], BF16, kind="ExternalInput")
    id_h = nc.dram_tensor("idm", [128, 128], BF16, kind="ExternalInput")
    lab_h = nc.dram_tensor("lab", [128, FLAB], U32, kind="ExternalInput")

    # ---- external outputs ----
    img_h = nc.dram_tensor("img", [D, HC, W], F32, kind="ExternalOutput")
    labo_h = nc.dram_tensor("labo", [128, FLAB], I16, kind="ExternalOutput")

    with tile.TileContext(nc) as tc:
        from contextlib import ExitStack
        with ExitStack() as _stk:
            cst = _stk.enter_context(tc.tile_pool(name="consts", bufs=1))
            sxp = _stk.enter_context(tc.tile_pool(name="sxp", bufs=2))
            cbp = _stk.enter_context(tc.tile_pool(name="cbp", bufs=2))
            ebp = _stk.enter_context(tc.tile_pool(name="ebp", bufs=8))
            xhp = _stk.enter_context(tc.tile_pool(name="xhp", bufs=2))
            zwp = _stk.enter_context(tc.tile_pool(name="zwp", bufs=2))
            zvp = _stk.enter_context(tc.tile_pool(name="zvp", bufs=2))
            ybp = _stk.enter_context(tc.tile_pool(name="ybp", bufs=1))
            zdp = _stk.enter_context(tc.tile_pool(name="zdp", bufs=2))
            zip_ = _stk.enter_context(tc.tile_pool(name="zip", bufs=2))
            ltp = _stk.enter_context(tc.tile_pool(name="ltp", bufs=2))
            labp = _stk.enter_context(tc.tile_pool(name="labp", bufs=1))
            labq = _stk.enter_context(tc.tile_pool(name="labq", bufs=2))
            psbp = _stk.enter_context(tc.tile_pool(name="psb", bufs=2, space="PSUM"))
            pshp = psbp
            pstp = _stk.enter_context(tc.tile_pool(name="pst", bufs=2, space="PSUM"))
            pswp = _stk.enter_context(tc.tile_pool(name="psw", bufs=2, space="PSUM"))
            drp = _stk.enter_context(tc.tile_pool(name="dram", bufs=1, space="DRAM"))
            # ---- constants ----
            ght = cst.tile([HS, HC], BF16)
            nc.sync.dma_start(ght[:], gh_h.ap())
            gwa = cst.tile([128, 96], BF16)
            nc.sync.dma_start(gwa[:], gwa_h.ap())
            gwc = cst.tile([128, 96], BF16)
            nc.sync.dma_start(gwc[:], gwc_h.ap())
            gda = cst.tile([128, 96], BF16)
            nc.sync.dma_start(gda[:], gda_h.ap())
            gdc = cst.tile([128, 96], BF16)
            nc.sync.dma_start(gdc[:], gdc_h.ap())
            whtt = cst.tile([4, HS], BF16)
            nc.sync.dma_start(whtt[:], wht_h.ap())
            idt = cst.tile([128, 128], BF16)
            nc.sync.dma_start(idt[:], id_h.ap())

            # y2 SBUF-resident: [w'-half, (d, h')] bf16, written by stage A
            ybA = ybp.tile([96, D * HC], BF16)
            ybC = ybp.tile([96, D * HC], BF16)

            lab_done = 0
            pending = []
            pending_img = []
            carry = []
            # prefetch two label chunks before stage A (DVE idle at startup)
            for _ in range(2):
                _emit_label_chunk(nc, ltp, labp, labq, lab_h, labo_h,
                                  lab_done, masks, pending, carry)
                lab_done += 1

            # ================= stage A =================
            for ib in range(NB_A):
                d0 = ib * DB
                sx = sxp.tile([HS, FA], BF16, tag="sx")
                nc.sync.dma_start(
                    sx[:],
                    bass.AP(xs_h, d0 * HS * W, [[W, HS], [HS * W, DB], [1, W]]),
                )
                cb = cbp.tile([4, FA], BF16, tag="cb")
                nc.sync.dma_start(cb[:], c_h.ap()[:, d0 * W:(d0 + DB) * W])

                xh = xhp.tile([HC, FA], BF16, tag="xh")
                NQ = FA // 512
                ebs_ = []
                for q in range(NQ):
                    sl = slice(q * 512, (q + 1) * 512)
                    psb = psbp.tile([HS, 512], F32, tag="psbh")
                    nc.tensor.matmul(psb[:], whtt[:], cb[:, sl], start=True, stop=True)
                    eb = ebp.tile([HS, 512], BF16, tag="eb")
                    nc.scalar.activation(eb[:], psb[:], mybir.ActivationFunctionType.Exp)
                    ebs_.append(eb)
                for q in range(NQ):
                    sl = slice(q * 512, (q + 1) * 512)
                    nc.gpsimd.tensor_tensor(sx[:, sl], sx[:, sl], ebs_[q][:], ALU.mult)
                pshs = []
                for q in range(NQ):
                    sl = slice(q * 512, (q + 1) * 512)
                    psh = pshp.tile([HC, 512], F32, tag="psbh")
                    nc.tensor.matmul(psh[:], ght[:], sx[:, sl], start=True, stop=True)
                    pshs.append(psh)
                for q in range(NQ):
                    sl = slice(q * 512, (q + 1) * 512)
                    nc.scalar.copy(xh[:, sl], pshs[q][:])

                # T1: w onto partitions; zwA rows 0..127, zwC rows 64..191
                # layout [128, (dl, h')]
                zwA = zwp.tile([128, DB * HC], BF16, tag="zwA")
                zwC = zwp.tile([128, DB * HC], BF16, tag="zwC")
                for g in range(DB // 8):
                    ptA = pstp.tile([128, 8 * HC], BF16, tag="pt")
                    ptC = pstp.tile([128, 8 * HC], BF16, tag="pt")
                    for t in range(8):
                        dl = g * 8 + t
                        nc.tensor.transpose(
                            ptA[:, t * HC:(t + 1) * HC],
                            xh[:, dl * W: dl * W + 128], idt[0:HC, 0:HC])
                        nc.tensor.transpose(
                            ptC[:, t * HC:(t + 1) * HC],
                            xh[:, dl * W + 64: dl * W + 192], idt[0:HC, 0:HC])
                    nc.scalar.copy(zwA[:, g * 8 * HC:(g + 1) * 8 * HC], ptA[:])
                    nc.scalar.copy(zwC[:, g * 8 * HC:(g + 1) * 8 * HC], ptC[:])

                # W-blur: m=0 from zwA (w rows 0..127), m=1 from zwC (64..191)
                for m, (gw_, zw_, yb_) in enumerate(((gwa, zwA, ybA), (gwc, zwC, ybC))):
                    for q in range(DB * HC // 512):
                        sl = slice(d0 * HC + q * 512, d0 * HC + (q + 1) * 512)
                        psw = pswp.tile([96, 512], F32, tag="psw")
                        nc.tensor.matmul(psw[:], gw_[:], zw_[:, q * 512:(q + 1) * 512],
                                         start=True, stop=True)
                        nc.scalar.copy(yb_[:, sl], psw[:])

                if lab_done < NB_A:
                    _emit_label_chunk(nc, ltp, labp, labq, lab_h, labo_h,
                                      lab_done, masks, pending, carry)
                    lab_done += 1

            # ================= stage B ================= (yb tiles already filled)

            for jb in range(NB_B):
                h0 = jb * HB
                # T2: d onto partitions; strided moving reads from yb tiles
                # zdA rows d 0..127, zdC rows d 64..191; layout [*, (hl, w)]
                zdA = zdp.tile([128, HB * W], BF16, tag="zdA")
                zdC = zdp.tile([128, HB * W], BF16, tag="zdC")
                for g in range(HB // 4):
                    ptA = pstp.tile([128, 768], BF16, tag="pt")
                    ptC = pstp.tile([128, 768], BF16, tag="pt")
                    for t in range(4):
                        hl = g * 4 + t
                        colA = bass.AP(ybA.tensor, ybA[:].offset + (h0 + hl),
                                       [ybA[:].ap[0], [HC, 128]])
                        colB = bass.AP(ybC.tensor, ybC[:].offset + (h0 + hl),
                                       [ybC[:].ap[0], [HC, 128]])
                        nc.tensor.transpose(ptA[:, t * 192 + 0:t * 192 + 96],
                                            colA, idt[0:96, 0:96])
                        nc.tensor.transpose(ptA[:, t * 192 + 96:t * 192 + 192],
                                            colB, idt[0:96, 0:96])
                        colA2 = bass.AP(ybA.tensor, ybA[:].offset + (h0 + hl) + 64 * HC,
                                        [ybA[:].ap[0], [HC, 128]])
                        colB2 = bass.AP(ybC.tensor, ybC[:].offset + (h0 + hl) + 64 * HC,
                                        [ybC[:].ap[0], [HC, 128]])
                        nc.tensor.transpose(ptC[:, t * 192 + 0:t * 192 + 96],
                                            colA2, idt[0:96, 0:96])
                        nc.tensor.transpose(ptC[:, t * 192 + 96:t * 192 + 192],
                                            colB2, idt[0:96, 0:96])
                    nc.scalar.copy(zdA[:, g * 768:(g + 1) * 768], ptA[:])
                    nc.scalar.copy(zdC[:, g * 768:(g + 1) * 768], ptC[:])

                # D-blur: n=0 -> img rows 0..95 from zdA; n=1 -> 96..191 from zdC
                for n, (gd_, zd_) in enumerate(((gda, zdA), (gdc, zdC))):
                    zi = zip_.tile([96, HB * W], F32, tag="zi")
                    for q in range(HB * W // 512):
                        sl = slice(q * 512, (q + 1) * 512)
                        psd = pswp.tile([96, 512], F32, tag="psw")
                        nc.tensor.matmul(psd[:], gd_[:], zd_[:, sl], start=True, stop=True)
                        nc.scalar.copy(zi[:, sl], psd[:])
                    pending_img.append((
                        bass.AP(img_h, n * 96 * HC * W + h0 * W,
                                [[HC * W, 96], [1, HB * W]]),
                        zi[:],
                    ))
                while len(pending_img) > 2:
                    po, pz = pending_img.pop(0)
                    nc.sync.dma_start(po, pz)

                if jb % 2 == 0 and lab_done < NLC:
                    _emit_label_chunk(nc, ltp, labp, labq, lab_h, labo_h,
                                      lab_done, masks, pending, carry)
                    lab_done += 1

            while lab_done < NLC:
                _emit_label_chunk(nc, ltp, labp, labq, lab_h, labo_h,
                                  lab_done, masks, pending, carry)
                lab_done += 1
            _flush_label_carry(nc, labo_h, pending, carry)
            while pending:
                po, pacc = pending.pop(0)
                nc.sync.dma_start(po, pacc)
            while pending_img:
                po, pz = pending_img.pop(0)
                nc.sync.dma_start(po, pz)

    nc.compile()
    return nc


def _host_prep(x, small_bias, sigma01, labels, source_values, dest_values):
    Wd = _lin_weights(SMALL, D)
    Whm = _lin_weights(SMALL, H)
    Wwm = _lin_weights(SMALL, W)
    eye_bf = np.eye(128, dtype=ml_dtypes.bfloat16)

    mapping = np.zeros(TABLE, np.int64)
    mapping[np.asarray(source_values, np.int64)] = np.asarray(dest_values, np.int64)

    x_np = np.asarray(x, np.float32)
    lab_np = np.asarray(labels)

    in_maps = []
    for c in range(8):
        b, half = c // 2, c % 2
        h0 = half * HC
        hidx = np.clip(np.arange(h0 - P, h0 + HC + P), 0, H - 1)

        xs = np.ascontiguousarray(x_np[b, 0][:, hidx, :]).astype(ml_dtypes.bfloat16)

        sm = np.asarray(small_bias[b, 0], np.float64) * BIAS_STD
        Cydw = np.einsum("xyz,dx,wz->ydw", sm, Wd, Wwm).reshape(4, D * W)
        WhT = np.ascontiguousarray(Whm[hidx, :].T)

        g3 = _gauss_kernels(np.asarray(sigma01[b], np.float64) * MAX_SIGMA)
        Gh = _slab_toeplitz(g3[1])
        Gw = _edge_folded_toeplitz(g3[2], W)
        Gd = _edge_folded_toeplitz(g3[0], D)

        lab = (np.uint32(1) << lab_np[b, 0][:, h0:h0 + HC, :].astype(np.uint32)
               ).reshape(128, FLAB)

        in_maps.append({
            "xs": xs,
            "cydw": Cydw.astype(ml_dtypes.bfloat16),
            "wht": WhT.astype(ml_dtypes.bfloat16),
            "gh": Gh.astype(ml_dtypes.bfloat16),
            "gwa": np.ascontiguousarray(Gw[0:128, 0:96]).astype(ml_dtypes.bfloat16),
            "gwc": np.ascontiguousarray(Gw[64:192, 96:192]).astype(ml_dtypes.bfloat16),
            "gda": np.ascontiguousarray(Gd[0:128, 0:96]).astype(ml_dtypes.bfloat16),
            "gdc": np.ascontiguousarray(Gd[64:192, 96:192]).astype(ml_dtypes.bfloat16),
            "idm": eye_bf,
            "lab": np.ascontiguousarray(lab),
        })
    return in_maps


def _label_masks(source_values, dest_values):
    mapping = np.zeros(TABLE, np.int64)
    mapping[np.asarray(source_values, np.int64)] = np.asarray(dest_values, np.int64)
    masks = []
    for b in range(7):
        m = 0
        for l in range(N_LABELS):
            if (int(mapping[l]) >> b) & 1:
                m |= 1 << l
        masks.append(m)
    return tuple(masks)


def kernel(x, small_bias, sigma01, labels, source_values, dest_values):
    masks = _label_masks(source_values, dest_values)
    if _CACHE.get("masks") != masks:
        _CACHE["nc"] = _build_program(masks)
        _CACHE["masks"] = masks
    nc = _CACHE["nc"]

    in_maps = _host_prep(x, small_bias, sigma01, labels, source_values, dest_values)
    res = run_bass_kernel_spmd(nc, in_maps, core_ids=list(range(8)))

    img = np.empty((B, C, D, H, W), np.float32)
    labels_out = np.empty((B, C, D, H, W), np.int32)
    for c in range(8):
        b, half = c // 2, c % 2
        h0 = half * HC
        r = res.results[c]
        img[b, 0, :, h0:h0 + HC, :] = r["img"].reshape(D, HC, W)
        labels_out[b, 0, :, h0:h0 + HC, :] = (
            r["labo"].reshape(D, HC, W).astype(np.int32))
    return img, labels_out
